# revision 1
# baseline (speedup 1.0000x reference)
"""Trainium2 Bass kernel for nn_CRAU (per-channel sparse attention).

Computation (per batch b, channel c):
  qc  = Wq @ src (1x1 conv; bias folded into the S-reduction seed)
  S[c,t] = sum_d unfold(qc)[c,t,d] * feat[c,d] * (1/64)      t in 3x3 window
  A   = softmax_t(S)
  vc  = Wv @ feat + bv (1x1 conv)
  out = fold(A outer vc) * src

Sharding: 8 cores = 4 batches x 2 spatial halves (rows). The q.k reduction
is spatially partial per core; a pairwise AllReduce of S ([128,9] f32 per
channel-half, issued as soon as that half's partials finish so softmax/fold
of one half overlaps the reduction/compute of the other) links the two
halves of each batch.

The fold/unfold (3x3, stride 2, pad 1) is decomposed into 4 output parity
classes, each a small per-channel linear combination of shifted vc planes,
executed with fused DVE scalar_tensor_tensor ops, ScalarE per-partition-
scale multiplies, and GpSimd tensor-tensor multiplies. Convs run on the PE
in fp16 (full rate); the q.k reduction uses the custom-DVE
TENSOR_TENSOR_REDUCE op reading a column-parity-split qc so most of the 9
window offsets stream with unit stride.
"""

import numpy as np

N_CORES = 8
SRC_R, SRC_C = 65, 129          # per-core src slab (padded rows/cols)
FEAT_R, FEAT_C = 33, 66         # per-core feat slab (padded, even width)
SRCN = SRC_R * SRC_C            # 8385
FEATN = FEAT_R * FEAT_C         # 2178
QE_C, QO_C = 66, 64             # qc even-col / odd-col tile widths
OUTN = 64 * 128                 # per-core output elements per channel
SCALE = 1.0 / 64.0
QROWS = 13                      # q-conv rows per PSUM chunk (13*129=1677)

_prog_cache = {}
TRACE = False
TRACE_KW = {}
LAST_RESULT = [None]
STAGE = [99]


def _build(add_bv: bool, stage: int = 99):
    import concourse.mybir as mybir
    import concourse.tile as tile
    from concourse import bacc
    from concourse.dve_ops import TENSOR_TENSOR_REDUCE

    f32 = mybir.dt.float32
    f16 = mybir.dt.float16
    ADD = mybir.AluOpType.add
    MULT = mybir.AluOpType.mult
    MAX = mybir.AluOpType.max
    AX = mybir.AxisListType.X
    Exp = mybir.ActivationFunctionType.Exp

    nc = bacc.Bacc("TRN2", target_bir_lowering=False, debug=False,
                   num_devices=N_CORES)

    src_d = nc.dram_tensor("src", [256, SRCN], f32, kind="ExternalInput").ap()
    feat_d = nc.dram_tensor("feat", [256, FEATN], f32, kind="ExternalInput").ap()
    wpack_d = nc.dram_tensor("wpack", [256, 512], f32, kind="ExternalInput").ap()
    sinit_d = nc.dram_tensor("s_init", [256, 9], f32, kind="ExternalInput").ap()
    bv_d = nc.dram_tensor("bv", [256, 1], f32, kind="ExternalInput").ap()
    out_d = nc.dram_tensor("out", [256, OUTN], f32, kind="ExternalOutput").ap()

    with tile.TileContext(nc) as tc:
        with (
            tc.tile_pool(name="srcp", bufs=2) as srcp,
            tc.tile_pool(name="featp", bufs=2) as featp,
            tc.tile_pool(name="vcp", bufs=2) as vcp,
            tc.tile_pool(name="qcp", bufs=1) as qcp,
            tc.tile_pool(name="constp", bufs=2) as constp,
            tc.tile_pool(name="smp", bufs=1) as smp,
            tc.tile_pool(name="tup", bufs=4) as tup,
            tc.tile_pool(name="outp", bufs=2) as outp,
            tc.tile_pool(name="ps", bufs=2, space="PSUM") as ps,
            tc.tile_pool(name="dramp", bufs=2, space="DRAM") as dramp,
        ):
            # ---- loads (chunked so compute starts early) ----
            src_t = []
            feat_t = []
            w_t = []
            for h in range(2):
                wt = constp.tile([128, 512], f16, tag="w")
                nc.gpsimd.dma_start(wt[:], wpack_d[128 * h:128 * h + 128, :])
                w_t.append(wt)
            for h in range(2):
                st = srcp.tile([128, SRCN], f16, tag="src")
                for c0 in range(0, SRCN, 2145):
                    csz = min(2145, SRCN - c0)
                    nc.gpsimd.dma_start(
                        st[:, c0:c0 + csz],
                        src_d[128 * h:128 * h + 128, c0:c0 + csz])
                src_t.append(st)
                ft = featp.tile([128, FEATN], f16, tag="feat")
                for c0 in range(0, FEATN, 1089):
                    nc.gpsimd.dma_start(
                        ft[:, c0:c0 + 1089],
                        feat_d[128 * h:128 * h + 128, c0:c0 + 1089])
                feat_t.append(ft)

            # smalls layout (cols):
            # [0:9] S(h0) [9:18] S(h1) [18:27] Ssum(h0) [27:36] Ssum(h1)
            # [36:45] A(h0) [45:54] A(h1) [54:63] E scratch
            # [63:64] m [64:65] nm [65:66] sum [66:67] r
            # [68:77] sinit(h0) [77:86] sinit(h1)  [86:88] bv(h0,h1)
            sm = smp.tile([128, 96], f32, tag="smalls")
            for h in range(2):
                nc.sync.dma_start(sm[:, 68 + 9 * h:77 + 9 * h],
                                  sinit_d[128 * h:128 * h + 128, :])
                if add_bv:
                    nc.sync.dma_start(sm[:, 86 + h:87 + h],
                                      bv_d[128 * h:128 * h + 128, :])

            # ---- v-conv (frees PSUM early; vc needed only for fold) ----
            vc_t = []
            for h in range(2 if stage >= 1 else 0):
                vt = vcp.tile([128, FEATN], f16, tag="vc")
                for c0 in range(0, FEATN, 2048):
                    csz = min(2048, FEATN - c0)
                    pt = ps.tile([128, 2048], f32, tag="mm")
                    for s0 in range(0, csz, 512):
                        ssz = min(512, csz - s0)
                        for kt in range(2):
                            nc.tensor.matmul(
                                pt[:, s0:s0 + ssz],
                                lhsT=w_t[kt][:, 256 + 128 * h:256 + 128 * h + 128],
                                rhs=feat_t[kt][:, c0 + s0:c0 + s0 + ssz],
                                start=(kt == 0), stop=(kt == 1))
                    if add_bv:
                        nc.vector.tensor_scalar(
                            out=vt[:, c0:c0 + csz], in0=pt[:, 0:csz],
                            scalar1=sm[:, 86 + h:87 + h], scalar2=None,
                            op0=ADD)
                    else:
                        nc.scalar.copy(vt[:, c0:c0 + csz], pt[:, 0:csz])
                if add_bv:
                    v3 = vt.rearrange("p (r q) -> p r q", q=FEAT_C)
                    nc.gpsimd.memset(v3[:, FEAT_R - 1, :], 0.0)
                    nc.gpsimd.memset(v3[:, :, 64:66], 0.0)
                vc_t.append(vt)

            # ---- q-conv + S partials + per-half collective ----
            S_b = []
            S_r = []
            for h in range(2):
                sbt = dramp.tile([128, 9], f32, tag=f"sb{h}", name=f"sb{h}")
                srt = dramp.tile([128, 9], f32, tag=f"sr{h}", name=f"sr{h}")
                S_b.append(sbt)
                S_r.append(srt)
            for h in range(2 if stage >= 2 else 0):
                # qc column-parity-split tiles:
                #   qe[r, m] = qc[r, 2m]   (m in [0,65), row width QE_C=66)
                #   qo[r, m] = qc[r, 2m+1] (m in [0,64))
                qe = qcp.tile([128, SRC_R * QE_C], f16, tag="qe")
                qo = qcp.tile([128, SRC_R * QO_C], f16, tag="qo")
                qe3 = qe.rearrange("p (r q) -> p r q", q=QE_C)
                qo3 = qo.rearrange("p (r q) -> p r q", q=QO_C)
                # row-aligned PSUM chunks of QROWS src rows each
                for r0 in range(0, SRC_R, QROWS):
                    nrow = min(QROWS, SRC_R - r0)
                    csz = nrow * SRC_C
                    c0 = r0 * SRC_C
                    pt = ps.tile([128, 2048], f32, tag="mm")
                    for s0 in range(0, csz, 512):
                        ssz = min(512, csz - s0)
                        for kt in range(2):
                            nc.tensor.matmul(
                                pt[:, s0:s0 + ssz],
                                lhsT=w_t[kt][:, 128 * h:128 * h + 128],
                                rhs=src_t[kt][:, c0 + s0:c0 + s0 + ssz],
                                start=(kt == 0), stop=(kt == 1))
                    pt3 = pt[:, 0:csz].rearrange("p (r q) -> p r q", q=SRC_C)
                    nc.scalar.copy(qe3[:, r0:r0 + nrow, 0:65],
                                   pt3[:, :, 0:129:2])
                    nc.scalar.copy(qo3[:, r0:r0 + nrow, 0:64],
                                   pt3[:, :, 1:128:2])

                k3 = feat_t[h].rearrange("p (r q) -> p r q", q=FEAT_C)
                scr = tup.tile([128, 2048], f16, tag="tu")
                scr3 = scr.rearrange("p (r q) -> p r q", q=64)
                for i in range(3):
                    for j in range(3):
                        t = 3 * i + j
                        if j == 0:
                            in0 = qe3[:, i:i + 63:2, 0:64]
                        elif j == 2:
                            in0 = qe3[:, i:i + 63:2, 1:65]
                        else:
                            in0 = qo3[:, i:i + 63:2, 0:64]
                        nc.vector._custom_dve(
                            TENSOR_TENSOR_REDUCE,
                            out=scr3[:],
                            in0=in0,
                            in1=k3[:, 0:32, 0:64],
                            s0=sm[:, 68 + 9 * h + t:69 + 9 * h + t],
                            s1=SCALE,
                            accum_out=sm[:, 9 * h + t:9 * h + t + 1])
                nc.sync.dma_start(S_b[h][:], sm[:, 9 * h:9 * h + 9])
                if stage >= 3:
                    nc.gpsimd.collective_compute(
                        "AllReduce", ADD,
                        replica_groups=[[0, 1], [2, 3], [4, 5], [6, 7]],
                        ins=[S_b[h].opt()], outs=[S_r[h].opt()])
                    nc.sync.dma_start(sm[:, 18 + 9 * h:27 + 9 * h], S_r[h][:])

            if stage == 2:
                for h in range(2):
                    nc.sync.dma_start(out_d[128 * h:128 * h + 128, 0:9],
                                      sm[:, 9 * h:9 * h + 9])
            if stage == 3:
                for h in range(2):
                    nc.sync.dma_start(out_d[128 * h:128 * h + 128, 0:9],
                                      sm[:, 18 + 9 * h:27 + 9 * h])

            # ---- softmax + fold + final multiply per half ----
            for h in range(2 if stage >= 4 else 0):
                Ss = sm[:, 18 + 9 * h:27 + 9 * h]
                Av = sm[:, 36 + 9 * h:45 + 9 * h]
                Ev = sm[:, 54:63]
                nc.vector.tensor_reduce(sm[:, 63:64], Ss, axis=AX, op=MAX)
                nc.scalar.mul(sm[:, 64:65], sm[:, 63:64], -1.0)
                nc.scalar.activation(Ev, Ss, Exp, bias=sm[:, 64:65], scale=1.0)
                nc.vector.tensor_reduce(sm[:, 65:66], Ev, axis=AX, op=ADD)
                nc.vector.reciprocal(sm[:, 66:67], sm[:, 65:66])
                nc.vector.tensor_scalar(out=Av, in0=Ev,
                                        scalar1=sm[:, 66:67], scalar2=None,
                                        op0=MULT)

                def a(t):
                    return Av[:, t:t + 1]

                if stage < 5:
                    nc.sync.dma_start(out_d[128 * h:128 * h + 128, 16:25],
                                      sm[:, 36 + 9 * h:45 + 9 * h])
                    continue

                vc3 = vc_t[h].rearrange("p (r q) -> p r q", q=FEAT_C)
                src3 = src_t[h].rearrange("p (r q) -> p r q", q=SRC_C)
                # whole-half views: out rows x in [0,64), v rows m in [0,33)
                v00 = vc3[:, 0:32, 0:64]
                v01 = vc3[:, 0:32, 1:65]
                v10 = vc3[:, 1:33, 0:64]
                v11 = vc3[:, 1:33, 1:65]
                s11 = src3[:, 1:64:2, 1:128:2]
                s12 = src3[:, 1:64:2, 2:129:2]
                s21 = src3[:, 2:65:2, 1:128:2]
                s22 = src3[:, 2:65:2, 2:129:2]

                O = outp.tile([128, OUTN], f32, tag="O")
                O3 = O.rearrange("p (x y) -> p x y", y=128)
                Oee = O3[:, 0:63:2, 0:127:2]
                Oeo = O3[:, 0:63:2, 1:128:2]
                Ooe = O3[:, 1:64:2, 0:127:2]
                Ooo = O3[:, 1:64:2, 1:128:2]

                def v2(tl):
                    return tl.rearrange("p (r q) -> p r q", q=64)

                # ee: (v00 * A4) * src
                nc.vector.scalar_tensor_tensor(
                    out=Oee, in0=v00, scalar=a(4), in1=s11,
                    op0=MULT, op1=MULT)
                # eo: (A3*v01 + A5*v00) * src
                T1 = tup.tile([128, 2048], f16, tag="tu")
                nc.scalar.mul(v2(T1), v00, a(5))
                U1 = tup.tile([128, 2048], f16, tag="tu")
                nc.vector.scalar_tensor_tensor(
                    out=v2(U1), in0=v01, scalar=a(3), in1=v2(T1),
                    op0=MULT, op1=ADD)
                nc.gpsimd.tensor_tensor(out=Oeo, in0=v2(U1), in1=s12, op=MULT)
                # oe: (A1*v10 + A7*v00) * src
                T2 = tup.tile([128, 2048], f16, tag="tu")
                nc.scalar.mul(v2(T2), v10, a(1))
                U2 = tup.tile([128, 2048], f16, tag="tu")
                nc.vector.scalar_tensor_tensor(
                    out=v2(U2), in0=v00, scalar=a(7), in1=v2(T2),
                    op0=MULT, op1=ADD)
                nc.gpsimd.tensor_tensor(out=Ooe, in0=v2(U2), in1=s21, op=MULT)
                # oo: (A0*v11 + A2*v10 + A6*v01 + A8*v00) * src
                T3 = tup.tile([128, 2048], f16, tag="tu")
                nc.scalar.mul(v2(T3), v11, a(0))
                T4 = tup.tile([128, 2048], f16, tag="tu")
                nc.scalar.mul(v2(T4), v01, a(6))
                U3 = tup.tile([128, 2048], f16, tag="tu")
                nc.vector.scalar_tensor_tensor(
                    out=v2(U3), in0=v10, scalar=a(2), in1=v2(T3),
                    op0=MULT, op1=ADD)
                U4 = tup.tile([128, 2048], f16, tag="tu")
                nc.vector.scalar_tensor_tensor(
                    out=v2(U4), in0=v00, scalar=a(8), in1=v2(T4),
                    op0=MULT, op1=ADD)
                U5 = tup.tile([128, 2048], f16, tag="tu")
                nc.vector.tensor_tensor(out=v2(U5), in0=v2(U3), in1=v2(U4),
                                        op=ADD)
                nc.gpsimd.tensor_tensor(out=Ooo, in0=v2(U5), in1=s22, op=MULT)

                nc.sync.dma_start(out_d[128 * h:128 * h + 128, :], O[:])

    nc.compile()
    return nc


def _get_program(add_bv: bool, stage: int = 99):
    key = (add_bv, stage)
    if key not in _prog_cache:
        _prog_cache[key] = _build(add_bv, stage)
    return _prog_cache[key]


def kernel(feat, src, Wq, bq, Wv, bv):
    from concourse.bass_utils import run_bass_kernel_spmd

    feat = np.ascontiguousarray(np.asarray(feat, dtype=np.float32))
    src = np.ascontiguousarray(np.asarray(src, dtype=np.float32))
    Wq = np.asarray(Wq, dtype=np.float32)
    bq = np.asarray(bq, dtype=np.float32)
    Wv = np.asarray(Wv, dtype=np.float32)
    bv = np.asarray(bv, dtype=np.float32)
    B, C, H, W = src.shape

    src_pad = np.pad(src, ((0, 0), (0, 0), (1, 1), (1, 1)))
    feat_pad = np.pad(feat, ((0, 0), (0, 0), (0, 1), (0, 2)))
    wpack = np.ascontiguousarray(
        np.concatenate([Wq.T, Wv.T], axis=1).astype(np.float32))

    add_bv = bool(np.any(bv))
    nc = _get_program(add_bv, STAGE[0])

    # bq correction seeds for the q.k reduction: S += bq * sum(valid k) * scale
    sinits = {}
    if np.any(bq):
        for b in range(B):
            for s in range(2):
                k = feat[b, :, 32 * s:32 * s + 32, :]
                corr = np.zeros((C, 9), np.float32)
                for i in range(3):
                    for j in range(3):
                        valid = np.ones((32, 64), bool)
                        if i == 0 and s == 0:
                            valid[0, :] = False
                        if j == 0:
                            valid[:, 0] = False
                        corr[:, 3 * i + j] = bq * (k * valid).sum((1, 2)) * SCALE
                sinits[(b, s)] = corr
    zero_sinit = np.zeros((C, 9), np.float32)

    in_maps = []
    for core in range(N_CORES):
        b, s = core // 2, core % 2
        src_slab = np.ascontiguousarray(
            src_pad[b, :, 64 * s:64 * s + SRC_R, :SRC_C].reshape(C, SRCN))
        feat_slab = np.ascontiguousarray(
            feat_pad[b, :, 32 * s:32 * s + FEAT_R, :FEAT_C].reshape(C, FEATN))
        in_maps.append({
            "src": src_slab,
            "feat": feat_slab,
            "wpack": wpack,
            "s_init": sinits.get((b, s), zero_sinit),
            "bv": bv.reshape(C, 1),
        })

    res = run_bass_kernel_spmd(nc, in_maps, list(range(N_CORES)),
                               trace=TRACE, **TRACE_KW)
    LAST_RESULT[0] = res

    out = np.empty((B, C, H, W), np.float32)
    for core in range(N_CORES):
        b, s = core // 2, core % 2
        out[b, :, 64 * s:64 * s + 64, :] = \
            res.results[core]["out"].reshape(C, 64, 128)
    return out



# revision 8
# speedup vs baseline: 1.7690x; 1.7690x over previous
"""Trainium2 Bass kernel for nn_CRAU (per-channel sparse attention).

Computation (per batch b, channel c):
  qc  = Wq @ src (1x1 conv; bq folded into the S-reduction seed)
  S[c,t] = sum_d unfold(qc)[c,t,d] * feat[c,d] * (1/64)      t in 3x3 window
  A   = softmax_t(S)
  vc  = Wv @ feat + bv (1x1 conv)
  out = fold(A outer vc) * src

Sharding: 8 cores = 4 batches x 2 output-channel groups of 128. The
attention is fully per-channel, so with channel-group sharding each core
owns the complete spatial reduction for its channels -- no collectives.
Each core's inputs are channel-permuted host-side (own group first) so the
SPMD program always works on partition rows 0..127.

Layout: all tensors f16 host-side; the padded src slab is packed as 4
row/col-parity quadrant planes [EE|EO|OE|OO] so the 9-offset q.k reduce,
the fold, and the final multiply all read unit-stride blocks. qc inherits
the quadrant layout from the matmul rhs. The fold runs on the PE as
diag(A_t) matmuls accumulating parity half-planes in PSUM; finals
(plane * src) are split between DVE (PSUM-direct) and GpSimd.
"""

import numpy as np

N_CORES = 8
SRC_R, SRC_C = 129, 129         # per-core padded src slab (full spatial)
FEAT_R, FEAT_C = 65, 66         # padded feat slab
# quadrant plane offsets in the packed src/qc layout
O_EE = 0
O_EO = O_EE + 65 * 65           # 4225
O_OE = O_EO + 65 * 64           # 8385
O_OO = O_OE + 64 * 65           # 12545
SRCN = O_OO + 64 * 64           # 16641
FEATN = FEAT_R * FEAT_C         # 4290
OUTN = 4 * 4096                 # 4 parity planes per channel
SCALE = 1.0 / 64.0

_prog_cache = {}
TRACE = False
TRACE_KW = {}
LAST_RESULT = [None]


def _build(add_bv: bool):
    import concourse.mybir as mybir
    import concourse.tile as tile
    from concourse import bacc
    from concourse.dve_ops import TENSOR_TENSOR_REDUCE

    f32 = mybir.dt.float32
    f16 = mybir.dt.float16
    ADD = mybir.AluOpType.add
    MULT = mybir.AluOpType.mult
    MAX = mybir.AluOpType.max
    AX = mybir.AxisListType.X
    Exp = mybir.ActivationFunctionType.Exp

    nc = bacc.Bacc("TRN2", target_bir_lowering=False, debug=False,
                   num_devices=N_CORES)

    src_d = nc.dram_tensor("src", [256, SRCN], f16, kind="ExternalInput").ap()
    feat_d = nc.dram_tensor("feat", [256, FEATN], f16, kind="ExternalInput").ap()
    wpack_d = nc.dram_tensor("wpack", [256, 256], f16, kind="ExternalInput").ap()
    eye_d = nc.dram_tensor("eye", [128, 128], f16, kind="ExternalInput").ap()
    sinit_d = nc.dram_tensor("s_init", [128, 9], f32, kind="ExternalInput").ap()
    bv_d = nc.dram_tensor("bv", [128, 1], f32, kind="ExternalInput").ap()
    out_d = nc.dram_tensor("out", [128, OUTN], f16, kind="ExternalOutput").ap()

    QCH = 2048
    q_chunks = []
    c0 = 0
    while c0 < SRCN:
        q_chunks.append((c0, min(QCH, SRCN - c0)))
        c0 += QCH

    with tile.TileContext(nc) as tc:
        with (
            tc.tile_pool(name="srcp", bufs=2) as srcp,
            tc.tile_pool(name="featp", bufs=2) as featp,
            tc.tile_pool(name="qcp", bufs=1) as qcp,
            tc.tile_pool(name="vcp", bufs=1) as vcp,
            tc.tile_pool(name="constp", bufs=2) as constp,
            tc.tile_pool(name="smp", bufs=1) as smp,
            tc.tile_pool(name="scrp", bufs=2) as scrp,
            tc.tile_pool(name="outp", bufs=4) as outp,
            tc.tile_pool(name="ps", bufs=2, space="PSUM") as ps,
        ):
            # ---- loads ----
            w_t = []
            for kt in range(2):
                wt = constp.tile([128, 256], f16, tag="w", name=f"w{kt}")
                nc.sync.dma_start(wt[:], wpack_d[128 * kt:128 * kt + 128, :])
                w_t.append(wt)
            eye = constp.tile([128, 128], f16, tag="eye")
            nc.sync.dma_start(eye[:], eye_d[:, :])

            # smalls: [0:9] S [9:18] A [18:27] E | [27] m [28] nm [29] sum
            # [30] r | [32:41] sinit [41:42] bv
            sm = smp.tile([128, 48], f32, tag="smalls")
            nc.sync.dma_start(sm[:, 32:41], sinit_d[:, :])
            if add_bv:
                nc.sync.dma_start(sm[:, 41:42], bv_d[:, :])

            src_t = [srcp.tile([128, SRCN], f16, tag="src", name=f"src{kt}")
                     for kt in range(2)]
            feat_t = [featp.tile([128, FEATN], f16, tag="feat",
                                 name=f"feat{kt}") for kt in range(2)]
            for ci, (c0, csz) in enumerate(q_chunks):
                for kt in range(2):
                    nc.gpsimd.dma_start(
                        src_t[kt][:, c0:c0 + csz],
                        src_d[128 * kt:128 * kt + 128, c0:c0 + csz])
                if ci == 0:
                    for kt in range(2):
                        nc.gpsimd.dma_start(
                            feat_t[kt][:],
                            feat_d[128 * kt:128 * kt + 128, :])

            # ---- q-conv (PE) + PSUM->SBUF f16 copies (ScalarE) ----
            qct = qcp.tile([128, SRCN], f16, tag="qc")
            for c0, csz in q_chunks:
                pt = ps.tile([128, 2048], f32, tag="mm")
                for kt in range(2):
                    for s0 in range(0, csz, 512):
                        ssz = min(512, csz - s0)
                        nc.tensor.matmul(
                            pt[:, s0:s0 + ssz],
                            lhsT=w_t[kt][:, 0:128],
                            rhs=src_t[kt][:, c0 + s0:c0 + s0 + ssz],
                            start=(kt == 0), stop=(kt == 1))
                nc.scalar.copy(qct[:, c0:c0 + csz], pt[:, 0:csz])

            # ---- v-conv (PE) + copies ----
            vt = vcp.tile([128, FEATN], f16, tag="vc")
            for c0 in range(0, FEATN, 2048):
                csz = min(2048, FEATN - c0)
                pt = ps.tile([128, 2048], f32, tag="mm")
                for kt in range(2):
                    for s0 in range(0, csz, 512):
                        ssz = min(512, csz - s0)
                        nc.tensor.matmul(
                            pt[:, s0:s0 + ssz],
                            lhsT=w_t[kt][:, 128:256],
                            rhs=feat_t[kt][:, c0 + s0:c0 + s0 + ssz],
                            start=(kt == 0), stop=(kt == 1))
                if add_bv:
                    nc.vector.tensor_scalar(
                        out=vt[:, c0:c0 + csz], in0=pt[:, 0:csz],
                        scalar1=sm[:, 41:42], scalar2=None, op0=ADD)
                else:
                    nc.scalar.copy(vt[:, c0:c0 + csz], pt[:, 0:csz])
            if add_bv:
                v3m = vt.rearrange("p (r q) -> p r q", q=FEAT_C)
                nc.gpsimd.memset(v3m[:, FEAT_R - 1, :], 0.0)
                nc.gpsimd.memset(v3m[:, :, 64:66], 0.0)

            # ---- q.k reduce (DVE custom op), quadrant order ----
            qEE = qct[:, O_EE:O_EO].rearrange("p (r q) -> p r q", q=65)
            qEO = qct[:, O_EO:O_OE].rearrange("p (r q) -> p r q", q=64)
            qOE = qct[:, O_OE:O_OO].rearrange("p (r q) -> p r q", q=65)
            qOO = qct[:, O_OO:SRCN].rearrange("p (r q) -> p r q", q=64)
            k3 = feat_t[0].rearrange("p (r q) -> p r q", q=FEAT_C)
            k64 = k3[:, 0:64, 0:64]
            scr = scrp.tile([128, 4096], f16, tag="scr")
            scr3 = scr.rearrange("p (r q) -> p r q", q=64)
            RED_ORDER = [(0, 0), (0, 2), (2, 0), (2, 2),
                         (0, 1), (2, 1), (1, 0), (1, 2), (1, 1)]
            for (i, j) in RED_ORDER:
                t = 3 * i + j
                rlo = 1 if i == 2 else 0
                if i == 1:
                    plane = qOO if j == 1 else qOE
                else:
                    plane = qEO if j == 1 else qEE
                clo = 1 if j == 2 else 0
                in0 = plane[:, rlo:rlo + 64, clo:clo + 64]
                nc.vector._custom_dve(
                    TENSOR_TENSOR_REDUCE,
                    out=scr3[:], in0=in0, in1=k64,
                    s0=sm[:, 32 + t:33 + t], s1=SCALE,
                    accum_out=sm[:, t:t + 1])

            # ---- softmax -> A ----
            Ss = sm[:, 0:9]
            Av = sm[:, 9:18]
            Ev = sm[:, 18:27]
            nc.vector.tensor_reduce(sm[:, 27:28], Ss, axis=AX, op=MAX)
            nc.scalar.mul(sm[:, 28:29], sm[:, 27:28], -1.0)
            nc.scalar.activation(Ev, Ss, Exp, bias=sm[:, 28:29], scale=1.0)
            nc.vector.tensor_reduce(sm[:, 29:30], Ev, axis=AX, op=ADD)
            nc.vector.reciprocal(sm[:, 30:31], sm[:, 29:30])
            nc.vector.tensor_scalar(out=Av, in0=Ev, scalar1=sm[:, 30:31],
                                    scalar2=None, op0=MULT)

            # diag(A_t) = eye * A[c,t]  (DVE tensor_scalar, 4x mode)
            dg = constp.tile([128, 9 * 128], f16, tag="dg")
            for t in range(9):
                nc.vector.tensor_scalar(
                    out=dg[:, 128 * t:128 * t + 128], in0=eye[:],
                    scalar1=Av[:, t:t + 1], scalar2=None, op0=MULT)

            # ---- fold: PE diag-matmuls into PSUM half-planes ----
            vc3 = vt.rearrange("p (r q) -> p r q", q=FEAT_C)
            sEE = src_t[0][:, O_EE:O_EO].rearrange("p (r q) -> p r q", q=65)
            sEO = src_t[0][:, O_EO:O_OE].rearrange("p (r q) -> p r q", q=64)
            sOE = src_t[0][:, O_OE:O_OO].rearrange("p (r q) -> p r q", q=65)
            sOO = src_t[0][:, O_OO:SRCN].rearrange("p (r q) -> p r q", q=64)

            # (A-term list [(t, dr, dc)], src multiplier plane slices)
            PLANES = [
                ([(4, 0, 0)], sOO),                       # out[2a,   2b]
                ([(3, 0, 1), (5, 0, 0)], sOE),            # out[2a,   2b+1]
                ([(1, 1, 0), (7, 0, 0)], sEO),            # out[2a+1, 2b]
                ([(0, 1, 1), (2, 1, 0), (6, 0, 1), (8, 0, 0)], sEE),
            ]
            SRC_SHIFT = [(0, 0), (0, 1), (1, 0), (1, 1)]  # src slice offsets
            for pi, (terms, spl) in enumerate(PLANES):
                sro, sco = SRC_SHIFT[pi]
                for hh in range(2):          # half-plane rows a in [32hh,+32)
                    pt = ps.tile([128, 2048], f32, tag="mm")
                    pt3 = pt.rearrange("p (r q) -> p r q", q=64)
                    nterm = len(terms)
                    for n, (t, dr, dc) in enumerate(terms):
                        for s0 in range(4):
                            r0 = 32 * hh + dr + 8 * s0
                            nc.tensor.matmul(
                                pt3[:, 8 * s0:8 * s0 + 8, :],
                                lhsT=dg[:, 128 * t:128 * t + 128],
                                rhs=vc3[:, r0:r0 + 8, dc:dc + 64],
                                start=(n == 0), stop=(n == nterm - 1))
                    ssl = spl[:, 32 * hh + sro:32 * hh + sro + 32,
                              sco:sco + 64]
                    ot = outp.tile([128, 2048], f16, tag="O")
                    ot3 = ot.rearrange("p (r q) -> p r q", q=64)
                    if pi % 2 == 0:
                        # DVE reads PSUM directly (1x), skips the copy
                        nc.vector.tensor_tensor(out=ot3[:], in0=pt3[:],
                                                in1=ssl, op=MULT)
                    else:
                        pf = scrp.tile([128, 2048], f16, tag="pf")
                        nc.scalar.copy(pf[:], pt[:, 0:2048])
                        pf3 = pf.rearrange("p (r q) -> p r q", q=64)
                        nc.gpsimd.tensor_tensor(out=ot3[:], in0=pf3[:],
                                                in1=ssl, op=MULT)
                    nc.sync.dma_start(
                        out_d[:, 4096 * pi + 2048 * hh:
                              4096 * pi + 2048 * hh + 2048], ot[:])

    nc.compile()
    return nc


def _get_program(add_bv: bool):
    if add_bv not in _prog_cache:
        _prog_cache[add_bv] = _build(add_bv)
    return _prog_cache[add_bv]


def _quad_pack(slab):
    """[C, 129, 129] -> [C, 16641] quadrant-packed [EE|EO|OE|OO]."""
    C = slab.shape[0]
    return np.concatenate([
        slab[:, 0::2, 0::2].reshape(C, -1),
        slab[:, 0::2, 1::2].reshape(C, -1),
        slab[:, 1::2, 0::2].reshape(C, -1),
        slab[:, 1::2, 1::2].reshape(C, -1),
    ], axis=1)


def kernel(feat, src, Wq, bq, Wv, bv):
    from concourse.bass_utils import run_bass_kernel_spmd

    feat = np.asarray(feat, dtype=np.float32)
    src = np.asarray(src, dtype=np.float32)
    Wq = np.asarray(Wq, dtype=np.float32)
    bq = np.asarray(bq, dtype=np.float32)
    Wv = np.asarray(Wv, dtype=np.float32)
    bv = np.asarray(bv, dtype=np.float32)
    B, C, H, W = src.shape

    src16 = np.pad(src, ((0, 0), (0, 0), (1, 1), (1, 1))).astype(np.float16)
    feat16 = np.pad(feat, ((0, 0), (0, 0), (0, 1), (0, 2))).astype(np.float16)
    eye = np.eye(128, dtype=np.float16)

    add_bv = bool(np.any(bv))
    nc = _get_program(add_bv)

    # bq correction seeds: S += bq * sum(valid k) * scale
    have_bq = bool(np.any(bq))
    zero_sinit = np.zeros((128, 9), np.float32)

    in_maps = []
    perms = []
    for core in range(N_CORES):
        b, g = core // 2, core % 2
        mine = slice(128 * g, 128 * g + 128)
        other = slice(128 * (1 - g), 128 * (1 - g) + 128)
        perm = np.r_[np.arange(128 * g, 128 * g + 128),
                     np.arange(128 * (1 - g), 128 * (1 - g) + 128)]
        perms.append((b, mine))
        src_slab = _quad_pack(src16[b][perm][:, :SRC_R, :SRC_C])
        feat_slab = feat16[b][perm].reshape(C, FEATN)
        wp = np.concatenate([Wq.T[:, mine], Wv.T[:, mine]], axis=1)[perm]
        if have_bq:
            k = feat[b, mine]
            corr = np.zeros((128, 9), np.float32)
            for i in range(3):
                for j in range(3):
                    valid = np.ones((64, 64), bool)
                    if i == 0:
                        valid[0, :] = False
                    if j == 0:
                        valid[:, 0] = False
                    corr[:, 3 * i + j] = \
                        bq[mine] * (k * valid).sum((1, 2)) * SCALE
            sinit = corr
        else:
            sinit = zero_sinit
        in_maps.append({
            "src": np.ascontiguousarray(src_slab),
            "feat": np.ascontiguousarray(feat_slab),
            "wpack": np.ascontiguousarray(wp.astype(np.float16)),
            "eye": eye,
            "s_init": sinit,
            "bv": bv[mine].reshape(128, 1),
        })

    res = run_bass_kernel_spmd(nc, in_maps, list(range(N_CORES)),
                               trace=TRACE, **TRACE_KW)
    LAST_RESULT[0] = res

    out = np.empty((B, C, H, W), np.float32)
    for core in range(N_CORES):
        b, mine = perms[core]
        o = res.results[core]["out"].astype(np.float32).reshape(128, 4, 64, 64)
        out[b, mine, 0::2, 0::2] = o[:, 0]
        out[b, mine, 0::2, 1::2] = o[:, 1]
        out[b, mine, 1::2, 0::2] = o[:, 2]
        out[b, mine, 1::2, 1::2] = o[:, 3]
    return out


# revision 13
# speedup vs baseline: 1.7966x; 1.0156x over previous
"""Trainium2 Bass kernel for nn_CRAU (per-channel sparse attention).

Computation (per batch b, channel c):
  qc  = Wq @ src (1x1 conv; bq folded into the S-reduction seed)
  S[c,t] = sum_d unfold(qc)[c,t,d] * feat[c,d] * (1/64)      t in 3x3 window
  A   = softmax_t(S)
  vc  = Wv @ feat + bv (1x1 conv)
  out = fold(A outer vc) * src

Sharding: 8 cores = 4 batches x 2 output-channel groups of 128. The
attention is fully per-channel, so with channel-group sharding each core
owns the complete spatial reduction for its channels -- no collectives.
Each core's inputs are channel-permuted host-side (own group first) so the
SPMD program always works on partition rows 0..127.

Layout: all tensors f16 host-side; the padded src slab is packed as 4
row/col-parity quadrant planes [EE|EO|OE|OO] so the 9-offset q.k reduce,
the fold, and the final multiply all read unit-stride blocks. qc inherits
the quadrant layout from the matmul rhs. The fold runs on the PE as
diag(A_t) matmuls accumulating parity half-planes in PSUM; finals
(plane * src) are split between DVE (PSUM-direct) and GpSimd.
"""

import numpy as np

N_CORES = 8
SRC_R, SRC_C = 129, 129         # per-core padded src slab (full spatial)
FEAT_R, FEAT_C = 65, 66         # padded feat slab
# quadrant plane offsets in the packed src/qc layout
O_EE = 0
O_EO = O_EE + 65 * 65           # 4225
O_OE = O_EO + 65 * 64           # 8385
O_OO = O_OE + 64 * 65           # 12545
SRCN = O_OO + 64 * 64           # 16641
FEATN = FEAT_R * FEAT_C         # 4290
OUTN = 4 * 4096                 # 4 parity planes per channel
SCALE = 1.0 / 64.0

_prog_cache = {}
TRACE = False
TRACE_KW = {}
LAST_RESULT = [None]


def _build(add_bv: bool):
    import concourse.mybir as mybir
    import concourse.tile as tile
    from concourse import bacc
    from concourse.dve_ops import TENSOR_TENSOR_REDUCE

    f32 = mybir.dt.float32
    f16 = mybir.dt.float16
    ADD = mybir.AluOpType.add
    MULT = mybir.AluOpType.mult
    MAX = mybir.AluOpType.max
    AX = mybir.AxisListType.X
    Exp = mybir.ActivationFunctionType.Exp

    nc = bacc.Bacc("TRN2", target_bir_lowering=False, debug=False,
                   num_devices=N_CORES)

    src_d = nc.dram_tensor("src", [256, SRCN], f16, kind="ExternalInput").ap()
    feat_d = nc.dram_tensor("feat", [256, FEATN], f16, kind="ExternalInput").ap()
    wpack_d = nc.dram_tensor("wpack", [256, 256], f16, kind="ExternalInput").ap()
    eye_d = nc.dram_tensor("eye", [128, 128], f16, kind="ExternalInput").ap()
    sinit_d = nc.dram_tensor("s_init", [128, 9], f32, kind="ExternalInput").ap()
    bv_d = nc.dram_tensor("bv", [128, 1], f32, kind="ExternalInput").ap()
    out_d = nc.dram_tensor("out", [128, OUTN], f16, kind="ExternalOutput").ap()

    QCH = 2048
    q_chunks = []
    c0 = 0
    while c0 < SRCN:
        q_chunks.append((c0, min(QCH, SRCN - c0)))
        c0 += QCH

    with tile.TileContext(nc) as tc:
        with (
            tc.tile_pool(name="srcp", bufs=2) as srcp,
            tc.tile_pool(name="featp", bufs=2) as featp,
            tc.tile_pool(name="qcp", bufs=1) as qcp,
            tc.tile_pool(name="vcp", bufs=1) as vcp,
            tc.tile_pool(name="constp", bufs=2) as constp,
            tc.tile_pool(name="smp", bufs=1) as smp,
            tc.tile_pool(name="scrp", bufs=2) as scrp,
            tc.tile_pool(name="outp", bufs=4) as outp,
            tc.tile_pool(name="ps", bufs=2, space="PSUM") as ps,
        ):
            # ---- loads ----
            w_t = []
            for kt in range(2):
                wt = constp.tile([128, 256], f16, tag="w", name=f"w{kt}")
                nc.sync.dma_start(wt[:], wpack_d[128 * kt:128 * kt + 128, :])
                w_t.append(wt)
            eye = constp.tile([128, 128], f16, tag="eye")
            nc.sync.dma_start(eye[:], eye_d[:, :])

            # smalls: [0:9] S [9:18] Ev [29] sum [30] r | [32:41] sinit
            # [41:42] bv
            sm = smp.tile([128, 48], f32, tag="smalls")
            nc.scalar.dma_start(sm[:, 32:41], sinit_d[:, :])
            if add_bv:
                nc.scalar.dma_start(sm[:, 41:42], bv_d[:, :])

            src_t = [srcp.tile([128, SRCN], f16, tag="src", name=f"src{kt}")
                     for kt in range(2)]
            feat_t = [featp.tile([128, FEATN], f16, tag="feat",
                                 name=f"feat{kt}") for kt in range(2)]
            # feat rides the sync queue in parallel with the src stream
            for kt in range(2):
                nc.sync.dma_start(feat_t[kt][:],
                                  feat_d[128 * kt:128 * kt + 128, :])
            for c0, csz in q_chunks:
                for kt in range(2):
                    nc.gpsimd.dma_start(
                        src_t[kt][:, c0:c0 + csz],
                        src_d[128 * kt:128 * kt + 128, c0:c0 + csz])

            # ---- q-conv (PE) + PSUM->SBUF f16 copies (ScalarE) ----
            qct = qcp.tile([128, SRCN], f16, tag="qc")
            for c0, csz in q_chunks:
                pt = ps.tile([128, 2048], f32, tag="mm")
                for kt in range(2):
                    for s0 in range(0, csz, 512):
                        ssz = min(512, csz - s0)
                        nc.tensor.matmul(
                            pt[:, s0:s0 + ssz],
                            lhsT=w_t[kt][:, 0:128],
                            rhs=src_t[kt][:, c0 + s0:c0 + s0 + ssz],
                            start=(kt == 0), stop=(kt == 1))
                nc.scalar.copy(qct[:, c0:c0 + csz], pt[:, 0:csz])

            # ---- v-conv (PE) + copies ----
            vt = vcp.tile([128, FEATN], f16, tag="vc")
            for c0 in range(0, FEATN, 2048):
                csz = min(2048, FEATN - c0)
                pt = ps.tile([128, 2048], f32, tag="mm")
                for kt in range(2):
                    for s0 in range(0, csz, 512):
                        ssz = min(512, csz - s0)
                        nc.tensor.matmul(
                            pt[:, s0:s0 + ssz],
                            lhsT=w_t[kt][:, 128:256],
                            rhs=feat_t[kt][:, c0 + s0:c0 + s0 + ssz],
                            start=(kt == 0), stop=(kt == 1))
                if add_bv:
                    nc.vector.tensor_scalar(
                        out=vt[:, c0:c0 + csz], in0=pt[:, 0:csz],
                        scalar1=sm[:, 41:42], scalar2=None, op0=ADD)
                else:
                    nc.scalar.copy(vt[:, c0:c0 + csz], pt[:, 0:csz])
            if add_bv:
                v3m = vt.rearrange("p (r q) -> p r q", q=FEAT_C)
                nc.gpsimd.memset(v3m[:, FEAT_R - 1, :], 0.0)
                nc.gpsimd.memset(v3m[:, :, 64:66], 0.0)

            # ---- q.k reduce (DVE custom op), quadrant order ----
            qEE = qct[:, O_EE:O_EO].rearrange("p (r q) -> p r q", q=65)
            qEO = qct[:, O_EO:O_OE].rearrange("p (r q) -> p r q", q=64)
            qOE = qct[:, O_OE:O_OO].rearrange("p (r q) -> p r q", q=65)
            qOO = qct[:, O_OO:SRCN].rearrange("p (r q) -> p r q", q=64)
            k3 = feat_t[0].rearrange("p (r q) -> p r q", q=FEAT_C)
            k64 = k3[:, 0:64, 0:64]
            scr = scrp.tile([128, 4096], f16, tag="scr")
            scr3 = scr.rearrange("p (r q) -> p r q", q=64)
            dg = constp.tile([128, 9 * 128], f16, tag="dg")
            # (i, j, row-range) — the first two EE ops are row-split so the
            # DVE starts before src chunk 2 lands; halves accumulate in place
            RED_ORDER = [(0, 0, 0, 32), (0, 2, 0, 32),
                         (0, 0, 32, 64), (0, 2, 32, 64),
                         (2, 0, 0, 64), (2, 2, 0, 64),
                         (0, 1, 0, 64), (2, 1, 0, 64),
                         (1, 0, 0, 64), (1, 2, 0, 64), (1, 1, 0, 64)]
            done = {}
            for (i, j, ra, rb) in RED_ORDER:
                t = 3 * i + j
                rlo = (1 if i == 2 else 0) + ra
                nrow = rb - ra
                if i == 1:
                    plane = qOO if j == 1 else qOE
                else:
                    plane = qEO if j == 1 else qEE
                clo = 1 if j == 2 else 0
                in0 = plane[:, rlo:rlo + nrow, clo:clo + 64]
                seed = sm[:, 32 + t:33 + t] if t not in done \
                    else sm[:, t:t + 1]
                nc.vector._custom_dve(
                    TENSOR_TENSOR_REDUCE,
                    out=scr3[:, 0:nrow, :], in0=in0, in1=k64[:, ra:rb, :],
                    s0=seed, s1=SCALE,
                    accum_out=sm[:, t:t + 1])
                done[t] = done.get(t, 0) + 1
                nparts = 2 if (i, j) in ((0, 0), (0, 2)) else 1
                if done[t] == nparts:
                    # S_t final: exp (no max-sub; |S|~O(6)) + diag(Ev_t)
                    nc.scalar.activation(sm[:, 9 + t:10 + t],
                                         sm[:, t:t + 1], Exp,
                                         bias=0.0, scale=1.0)
                    nc.vector.tensor_scalar(
                        out=dg[:, 128 * t:128 * t + 128], in0=eye[:],
                        scalar1=sm[:, 9 + t:10 + t], scalar2=None, op0=MULT)

            # normalization: r = 1/sum(Ev); applied in the finals
            nc.vector.tensor_reduce(sm[:, 29:30], sm[:, 9:18], axis=AX,
                                    op=ADD)
            nc.vector.reciprocal(sm[:, 30:31], sm[:, 29:30])

            # ---- fold: PE diag-matmuls into PSUM half-planes ----
            vc3 = vt.rearrange("p (r q) -> p r q", q=FEAT_C)
            sEE = src_t[0][:, O_EE:O_EO].rearrange("p (r q) -> p r q", q=65)
            sEO = src_t[0][:, O_EO:O_OE].rearrange("p (r q) -> p r q", q=64)
            sOE = src_t[0][:, O_OE:O_OO].rearrange("p (r q) -> p r q", q=65)
            sOO = src_t[0][:, O_OO:SRCN].rearrange("p (r q) -> p r q", q=64)

            # (A-term list [(t, dr, dc)], src multiplier plane slices)
            PLANES = [
                ([(4, 0, 0)], sOO),                       # out[2a,   2b]
                ([(3, 0, 1), (5, 0, 0)], sOE),            # out[2a,   2b+1]
                ([(1, 1, 0), (7, 0, 0)], sEO),            # out[2a+1, 2b]
                ([(0, 1, 1), (2, 1, 0), (6, 0, 1), (8, 0, 0)], sEE),
            ]
            SRC_SHIFT = [(0, 0), (0, 1), (1, 0), (1, 1)]  # src slice offsets
            for pi, (terms, spl) in enumerate(PLANES):
                sro, sco = SRC_SHIFT[pi]
                for hh in range(2):          # half-plane rows a in [32hh,+32)
                    pt = ps.tile([128, 2048], f32, tag="mm")
                    pt3 = pt.rearrange("p (r q) -> p r q", q=64)
                    nterm = len(terms)
                    for n, (t, dr, dc) in enumerate(terms):
                        for s0 in range(4):
                            r0 = 32 * hh + dr + 8 * s0
                            nc.tensor.matmul(
                                pt3[:, 8 * s0:8 * s0 + 8, :],
                                lhsT=dg[:, 128 * t:128 * t + 128],
                                rhs=vc3[:, r0:r0 + 8, dc:dc + 64],
                                start=(n == 0), stop=(n == nterm - 1))
                    ssl = spl[:, 32 * hh + sro:32 * hh + sro + 32,
                              sco:sco + 64]
                    ot = outp.tile([128, 2048], f16, tag="O")
                    ot3 = ot.rearrange("p (r q) -> p r q", q=64)
                    if pi % 2 == 0:
                        # DVE reads PSUM directly; r applied via the scalar
                        nc.vector.scalar_tensor_tensor(
                            out=ot3[:], in0=pt3[:], scalar=sm[:, 30:31],
                            in1=ssl, op0=MULT, op1=MULT)
                    else:
                        pf = scrp.tile([128, 2048], f16, tag="pf")
                        nc.scalar.mul(pf[:], pt[:, 0:2048], sm[:, 30:31])
                        pf3 = pf.rearrange("p (r q) -> p r q", q=64)
                        nc.gpsimd.tensor_tensor(out=ot3[:], in0=pf3[:],
                                                in1=ssl, op=MULT)
                    nc.sync.dma_start(
                        out_d[:, 4096 * pi + 2048 * hh:
                              4096 * pi + 2048 * hh + 2048], ot[:])

    nc.compile()
    return nc


def _get_program(add_bv: bool):
    if add_bv not in _prog_cache:
        _prog_cache[add_bv] = _build(add_bv)
    return _prog_cache[add_bv]


def _quad_pack(slab):
    """[C, 129, 129] -> [C, 16641] quadrant-packed [EE|EO|OE|OO]."""
    C = slab.shape[0]
    return np.concatenate([
        slab[:, 0::2, 0::2].reshape(C, -1),
        slab[:, 0::2, 1::2].reshape(C, -1),
        slab[:, 1::2, 0::2].reshape(C, -1),
        slab[:, 1::2, 1::2].reshape(C, -1),
    ], axis=1)


def kernel(feat, src, Wq, bq, Wv, bv):
    from concourse.bass_utils import run_bass_kernel_spmd

    feat = np.asarray(feat, dtype=np.float32)
    src = np.asarray(src, dtype=np.float32)
    Wq = np.asarray(Wq, dtype=np.float32)
    bq = np.asarray(bq, dtype=np.float32)
    Wv = np.asarray(Wv, dtype=np.float32)
    bv = np.asarray(bv, dtype=np.float32)
    B, C, H, W = src.shape

    src16 = np.pad(src, ((0, 0), (0, 0), (1, 1), (1, 1))).astype(np.float16)
    feat16 = np.pad(feat, ((0, 0), (0, 0), (0, 1), (0, 2))).astype(np.float16)
    eye = np.eye(128, dtype=np.float16)

    add_bv = bool(np.any(bv))
    nc = _get_program(add_bv)

    # bq correction seeds: S += bq * sum(valid k) * scale
    have_bq = bool(np.any(bq))
    zero_sinit = np.zeros((128, 9), np.float32)

    in_maps = []
    perms = []
    for core in range(N_CORES):
        b, g = core // 2, core % 2
        mine = slice(128 * g, 128 * g + 128)
        other = slice(128 * (1 - g), 128 * (1 - g) + 128)
        perm = np.r_[np.arange(128 * g, 128 * g + 128),
                     np.arange(128 * (1 - g), 128 * (1 - g) + 128)]
        perms.append((b, mine))
        src_slab = _quad_pack(src16[b][perm][:, :SRC_R, :SRC_C])
        feat_slab = feat16[b][perm].reshape(C, FEATN)
        wp = np.concatenate([Wq.T[:, mine], Wv.T[:, mine]], axis=1)[perm]
        if have_bq:
            k = feat[b, mine]
            corr = np.zeros((128, 9), np.float32)
            for i in range(3):
                for j in range(3):
                    valid = np.ones((64, 64), bool)
                    if i == 0:
                        valid[0, :] = False
                    if j == 0:
                        valid[:, 0] = False
                    corr[:, 3 * i + j] = \
                        bq[mine] * (k * valid).sum((1, 2)) * SCALE
            sinit = corr
        else:
            sinit = zero_sinit
        in_maps.append({
            "src": np.ascontiguousarray(src_slab),
            "feat": np.ascontiguousarray(feat_slab),
            "wpack": np.ascontiguousarray(wp.astype(np.float16)),
            "eye": eye,
            "s_init": sinit,
            "bv": bv[mine].reshape(128, 1),
        })

    res = run_bass_kernel_spmd(nc, in_maps, list(range(N_CORES)),
                               trace=TRACE, **TRACE_KW)
    LAST_RESULT[0] = res

    out = np.empty((B, C, H, W), np.float32)
    for core in range(N_CORES):
        b, mine = perms[core]
        o = res.results[core]["out"].astype(np.float32).reshape(128, 4, 64, 64)
        out[b, mine, 0::2, 0::2] = o[:, 0]
        out[b, mine, 0::2, 1::2] = o[:, 1]
        out[b, mine, 1::2, 0::2] = o[:, 2]
        out[b, mine, 1::2, 1::2] = o[:, 3]
    return out
